# revision 16
# baseline (speedup 1.0000x reference)
"""Butterfly-layer Trainium2 kernel.

Computation (per reference): input conv (8->64ch matmul + relu), 10 butterfly
levels (each branch spawns 2 children via kernel-2 stride-2 conv + relu), a
per-(branch,position) dense CxC layer in the middle, and a per-branch 64->8
output conv.

Sharding: data-parallel over batch across 8 cores (64 rows each), params
replicated.

Layout strategy per core (all activations bf16, fp32 PSUM accumulation):
- State tensors are always [128, 32768] bf16: partition = (half, channel),
  free = (pair, batch, position).
- Levels 1-7 ("scheme B"): contraction over channel (64) with two accumulating
  matmuls (taps i=0,1) reading stride-2 position slices; children pairs land
  stacked on partition halves -> full-width identity PSUM drains.
- Levels 0, 8, 9 ("scheme A"): contraction over (tap, channel) = 128 with both
  children packed in the 128 output columns; needs pair-split input layout
  (position parity on partition halves), produced by 4-way split drains.
- Mid dense: per (branch, position) CxC; branch pairs run as two concurrent
  32x32-granular diagonal tiles (0,0)/(64,64); outputs free-stacked in PSUM.
- Output conv: per branch-pair lhsT [128,16]; results DMA'd from PSUM to DRAM.
"""

import sys
import os

sys.path.insert(0, '/opt/trn_rl_repo')

import numpy as np
import ml_dtypes

import concourse.bass as bass
import concourse.bacc as bacc
import concourse.mybir as mybir
import concourse.tile as tile
from concourse import bass_utils

npbf = ml_dtypes.bfloat16
BF16 = mybir.dt.bfloat16
F32 = mybir.dt.float32
AF = mybir.ActivationFunctionType

NCORES = 8
B = 512
BC = B // NCORES          # 64 batch rows per core
C = 64
IN_F = 8
OUT_F = 8
NLVL = 10
L0 = 1 << NLVL            # 1024
KH = 1 << (NLVL // 2)     # 32
TOTAL_BR = (1 << (NLVL + 1)) - 2  # 2046


def _off(lvl):
    return (1 << (lvl + 1)) - 2


# ---------------------------------------------------------------------------
# Host-side weight packing
# ---------------------------------------------------------------------------

def _pack_weights(w_in, b_in, filters, biases, mid_w, mid_b, w_out):
    f32 = np.float32
    filters = np.asarray(filters, f32)
    w_in = np.asarray(w_in, f32)
    mid_w = np.asarray(mid_w, f32)
    w_out = np.asarray(w_out, f32)
    out = {}

    # identity for PE transpose
    out['ident'] = np.eye(64, dtype=npbf)

    # input conv: 8 block matmuls; lhsT [p=(l16,f)=128, (j,c)=128]
    win = np.zeros((16, 8, 8, 2, 64), f32)  # [l16, f, m, j, c]
    w = w_in[:, 0, :]  # [8, 64]
    for m in range(8):
        for j in range(2):
            win[2 * m + j, :, m, j, :] = w
    out['win'] = win.reshape(128, 8, 128).astype(npbf)

    # level 0 (scheme A, 1 parent): lhsT [(i,c), (j,o)]
    wa0 = filters[0:2].transpose(1, 2, 0, 3).reshape(128, 128)  # [i,c,j,o]
    out['wa0'] = np.ascontiguousarray(wa0).astype(npbf)

    # levels 1..7 (scheme B): [128=(s,c), K/2 pairs, 2 taps, 128=(j,o)]
    for lvl in range(1, 8):
        K = 1 << lvl
        off = _off(lvl)
        f = filters[off:off + 2 * K]          # [2K, 2, C, C] children of level
        # child of parent m, tap i: f[2m+j, i, c, o]
        f = f.reshape(K // 2, 2, 2, 2, C, C)  # [t, s, j, i, c, o]
        wb = f.transpose(1, 4, 0, 3, 2, 5)    # [s, c, t, i, j, o]
        wb = wb.reshape(128, K // 2, 2, 128)
        out[f'wb{lvl}'] = np.ascontiguousarray(wb).astype(npbf)

    # mid: [128=(s,c), 16 t, 32 x, 64 o]
    wm = mid_w.reshape(16, 2, KH, C, C).transpose(1, 3, 0, 2, 4)  # [s,c,t,x,o]
    out['wmid'] = np.ascontiguousarray(wm.reshape(128, 16, KH * C)).astype(npbf)

    # levels 8, 9 (scheme A): [128=(i,c), K parents, 128=(j,o)]
    for lvl in (8, 9):
        K = 1 << lvl
        off = _off(lvl)
        f = filters[off:off + 2 * K].reshape(K, 2, 2, C, C)  # [m, j, i, c, o]
        wa = f.transpose(2, 3, 0, 1, 4).reshape(128, K, 128)  # [(i,c), m, (j,o)]
        out[f'wa{lvl}'] = np.ascontiguousarray(wa).astype(npbf)

    # out conv: [128=(j,c), 512 pairs, 16=(j',o)]
    wo = np.zeros((2, C, 512, 2, OUT_F), f32)  # [j, c, m2, j', o]
    wr = w_out.reshape(512, 2, C, OUT_F)       # [m2, j, c, o]
    for j in range(2):
        wo[j, :, :, j, :] = wr[:, j, :, :].transpose(1, 0, 2)
    out['wout'] = np.ascontiguousarray(wo.reshape(128, 512, 16)).astype(npbf)

    # biases, packed per level as [128, K]: col m = [bias(2m) | bias(2m+1)]
    biases = np.asarray(biases, np.float32)
    bl = {}
    for lvl in range(NLVL):
        K = 1 << lvl
        off = _off(lvl)
        bb = biases[off:off + 2 * K].reshape(K, 2, C).transpose(1, 2, 0)
        bl[lvl] = np.ascontiguousarray(bb.reshape(128, K))
    out['bias_levels'] = bl
    out['b_in'] = np.broadcast_to(np.asarray(b_in, np.float32), (2, 64)).reshape(128, 1).copy()
    # mid bias [128=(s,o), 16 t, 32 x]
    bm = np.asarray(mid_b, np.float32).reshape(16, 2, KH, C).transpose(1, 3, 0, 2)
    out['bmid'] = np.ascontiguousarray(bm.reshape(128, 16, KH))
    return out


# ---------------------------------------------------------------------------
# Bass program
# ---------------------------------------------------------------------------

def _build_program(zero_bias_late, stop_after=99):
    """Build the per-core bass program. zero_bias_late: biases of levels 7-9
    and mid are all-zero -> use pooled (fast) drains there."""
    nc = bacc.Bacc('TRN2', target_bir_lowering=False, debug=False)

    dr = {}
    dr['x'] = nc.dram_tensor('x', [BC, 8192], BF16, kind='ExternalInput')
    dr['ident'] = nc.dram_tensor('ident', [64, 64], BF16, kind='ExternalInput')
    dr['win'] = nc.dram_tensor('win', [128, 8, 128], BF16, kind='ExternalInput')
    dr['wa0'] = nc.dram_tensor('wa0', [128, 128], BF16, kind='ExternalInput')
    for lvl in range(1, 8):
        K = 1 << lvl
        dr[f'wb{lvl}'] = nc.dram_tensor(f'wb{lvl}', [128, K // 2, 2, 128], BF16,
                                        kind='ExternalInput')
    dr['wmid'] = nc.dram_tensor('wmid', [128, 16, KH * C], BF16, kind='ExternalInput')
    dr['wa8'] = nc.dram_tensor('wa8', [128, 256, 128], BF16, kind='ExternalInput')
    dr['wa9'] = nc.dram_tensor('wa9', [128, 512, 128], BF16, kind='ExternalInput')
    dr['wout'] = nc.dram_tensor('wout', [128, 512, 16], BF16, kind='ExternalInput')
    for lvl in range(NLVL):
        dr[f'bl{lvl}'] = nc.dram_tensor(f'bl{lvl}', [128, 1 << lvl], F32,
                                        kind='ExternalInput')
    dr['b_in'] = nc.dram_tensor('b_in', [128, 1], F32, kind='ExternalInput')
    dr['bmid'] = nc.dram_tensor('bmid', [128, 16, KH], F32, kind='ExternalInput')
    dr['y'] = nc.dram_tensor('y', [BC, 8192], F32, kind='ExternalOutput')

    with tile.TileContext(nc) as tc:
        _emit(nc, tc, dr, zero_bias_late, stop_after)
    nc.compile()
    return nc


def _emit(nc, tc, dr, zero_bias_late, stop_after=99):
    from contextlib import ExitStack
    ctx = ExitStack()
    with ctx:
        states = ctx.enter_context(tc.tile_pool(name='state', bufs=2))
        misc = ctx.enter_context(tc.tile_pool(name='misc', bufs=1))
        wpool = ctx.enter_context(tc.tile_pool(name='wts', bufs=3))
        drains = 0  # round-robin ACT/DVE

        def drain(dst, src, bias=None, relu=True):
            nonlocal drains
            drains += 1
            if bias is None and not relu:
                if drains % 2 == 0:
                    nc.scalar.copy(dst, src)
                else:
                    nc.vector.tensor_copy(dst, src)
                return
            if bias is None and relu:
                if drains % 2 == 0:
                    nc.scalar.activation(dst, src, AF.Relu)
                else:
                    nc.vector.tensor_scalar_max(dst, src, 0.0)
                return
            nc.scalar.activation(dst, src, AF.Relu if relu else AF.Copy, bias=bias)

        # constants / biases resident in SBUF
        ident = misc.tile([64, 64], BF16)
        nc.sync.dma_start(ident[:], dr['ident'].ap())
        bt = {}
        for lvl in range(NLVL):
            bt[lvl] = misc.tile([128, 1 << lvl], F32, tag=f'bl{lvl}',
                                name=f'bl{lvl}_t')
            nc.sync.dma_start(bt[lvl][:], dr[f'bl{lvl}'].ap())
        b_in = misc.tile([128, 1], F32, tag='b_in')
        nc.sync.dma_start(b_in[:], dr['b_in'].ap())
        bmid = misc.tile([128, 16, KH], F32, tag='bmid')
        nc.sync.dma_start(bmid[:], dr['bmid'].ap())

        x_in = misc.tile([BC, 8192], BF16, tag='x_in')
        nc.sync.dma_start(x_in[:], dr['x'].ap())

        win = misc.tile([128, 8, 128], BF16, tag='win')
        nc.sync.dma_start(win[:], dr['win'].ap())
        wa0 = misc.tile([128, 128], BF16, tag='wa0')
        nc.sync.dma_start(wa0[:], dr['wa0'].ap())

        xt = misc.tile([128, 64, BC], BF16, tag='xt')  # [(l16,f), lhi, b]

        with tc.tile_pool(name='psA', bufs=6, space='PSUM') as psA:
            # ------- transpose input: 64 chunks of [64,128] -> [128,64]
            if stop_after < 0:
                return
            with nc.named_scope('transpose_in'):
                for g in range(8):
                    pt = psA.tile([128, 512], BF16, tag='ps')
                    for a in range(8):
                        j = g * 8 + a
                        nc.tensor.transpose(pt[:, a * 64:(a + 1) * 64],
                                            x_in[:, j * 128:(j + 1) * 128],
                                            ident[:])
                    drain(xt[:, g * 8:(g + 1) * 8, :], pt[:], relu=False)

            # ------- input conv -> V0 [(i,c), (b, q)]  q = 8*lhi + m
            if stop_after < 1:
                return
            V0 = states.tile([128, BC, 512], BF16, tag='state')
            v0q = V0[:].rearrange('p b (lhi m) -> p lhi b m', m=8)
            with nc.named_scope('in_conv'):
                for m in range(8):
                    for ch in range(8):
                        pt = psA.tile([128, 512], F32, tag='ps')
                        nc.tensor.matmul(pt[:], win[:, m, :],
                                         xt[:, ch * 8:(ch + 1) * 8, :]
                                         .rearrange('p l b -> p (l b)'),
                                         start=True, stop=True)
                        dst = v0q[:, ch * 8:(ch + 1) * 8, :, m]
                        drain(dst, pt[:], bias=b_in[:])

            # ------- level 0 (scheme A, one parent)
            if stop_after < 2:
                return
            V1 = states.tile([128, 1, BC, 512], BF16, tag='state')
            with nc.named_scope('level0'):
                for b0 in range(BC):
                    pt = psA.tile([128, 512], F32, tag='ps')
                    nc.tensor.matmul(pt[:], wa0[:], V0[:, b0, :],
                                     start=True, stop=True)
                    drain(V1[:, 0, b0, :], pt[:], bias=bt[0][:, 0:1])

            # ------- levels 1..4 (scheme B)
            Vcur = V1
            for lvl in range(1, 5):
                if stop_after < 2 + lvl:
                    return
                Vcur = _scheme_b_level(nc, tc, psA, states, wpool, dr, bt, lvl,
                                       Vcur, drain)

            # ------- mid dense (diag tiles)
            if stop_after < 7:
                return
            VM = Vcur  # [128, 16, BC, 32]
            MO = states.tile([128, 16, BC, KH], BF16, tag='state')
            wmt_shape = [128, KH * C]
            with nc.named_scope('mid'):
                for t in range(16):
                    wmt = wpool.tile(wmt_shape, BF16, tag='w')
                    nc.sync.dma_start(wmt[:], dr['wmid'].ap()[:, t, :])
                    wmv = wmt[:].rearrange('p (x o) -> p x o', o=C)
                    for xg in range(4):  # groups of 8 x positions
                        pt = psA.tile([128, 512], F32, tag='ps')
                        for a in range(8):
                            x = xg * 8 + a
                            cs = slice(a * 64, (a + 1) * 64)
                            nc.tensor.matmul(pt[0:64, cs], wmv[0:64, x, :],
                                             VM[0:64, t, :, x],
                                             start=True, stop=True,
                                             skip_group_check=True,
                                             tile_position=(0, 0))
                            nc.tensor.matmul(pt[64:128, cs], wmv[64:128, x, :],
                                             VM[64:128, t, :, x],
                                             start=True, stop=True,
                                             skip_group_check=True,
                                             tile_position=(64, 64))
                        mo_dst = MO[:, t, :, :].rearrange('p b x -> p x b')
                        if zero_bias_late:
                            drain(mo_dst[:, xg * 8:(xg + 1) * 8, :], pt[:])
                        else:
                            for a in range(8):
                                x = xg * 8 + a
                                drain(mo_dst[:, x:x + 1, :],
                                      pt[:, a * 64:(a + 1) * 64]
                                      .rearrange('p c -> p 1 c'),
                                      bias=bmid[:, t, x:x + 1])

            # ------- levels 5, 6 (scheme B)
            Vcur = MO
            for lvl in range(5, 7):
                if stop_after < 3 + lvl:
                    return
                Vcur = _scheme_b_level(nc, tc, psA, states, wpool, dr, bt, lvl,
                                       Vcur, drain)

        # ------- level 7 (scheme B matmuls, 4-way split drains -> A8)
        if stop_after < 10:
            return
        V7 = Vcur  # [128, 64, BC, 8]
        A8 = states.tile([128, 256, BC, 2], BF16, tag='state')
        with tc.tile_pool(name='psB', bufs=2, space='PSUM') as psB:
            with nc.named_scope('level7'):
                # tile_position matmuls require bank-aligned (512-col) psum
                # offsets -> 4 parents per 4-bank tile, 256 cols used each.
                wtile = None
                for mg in range(32):  # 4 parents per group
                    if mg % 4 == 0:
                        wtile = wpool.tile([128, 8, 2, 128], BF16, tag='w')
                        nc.sync.dma_start(
                            wtile[:], dr['wb7'].ap()[:, mg * 2:mg * 2 + 8, :, :])
                    pt = psB.tile([128, 2048], F32, tag='psb')
                    for p in range(4):
                        m = mg * 4 + p
                        t, s = m >> 1, m & 1
                        tt = (mg % 4) * 2 + (p >> 1)
                        sp = slice(s * 64, (s + 1) * 64)
                        rhs = V7[sp, t, :, :].rearrange('c b (q i) -> c b q i', i=2)
                        for i in range(2):
                            nc.tensor.matmul(
                                pt[:, p * 512:p * 512 + 256], wtile[sp, tt, i, :],
                                rhs[:, :, :, i].rearrange('c b q -> c (b q)'),
                                start=(i == 0), stop=(i == 1),
                                skip_group_check=True,
                                tile_position=(s * 64, 0))
                    # drains: psum [(j,o), (4 parents x 512, b, q=2q''+i)] -> A8
                    pv = pt[:].rearrange('p (pr blk) -> p pr blk', pr=4)[:, :, 0:256] \
                        .rearrange('p pr (b q2 i) -> p pr b q2 i', q2=2, i=2)
                    if zero_bias_late:
                        for j in range(2):
                            for i in range(2):
                                dst = A8[i * 64:(i + 1) * 64,
                                         2 * mg * 4 + j: 2 * (mg + 1) * 4: 2, :, :]
                                drain(dst, pv[j * 64:(j + 1) * 64, :, :, :, i])
                    else:
                        for p in range(4):
                            m = mg * 4 + p
                            for j in range(2):
                                for i in range(2):
                                    dst = A8[i * 64:(i + 1) * 64, 2 * m + j, :, :]
                                    drain(dst, pv[j * 64:(j + 1) * 64, p, :, :, i],
                                          bias=bt[7][j * 64:(j + 1) * 64, m:m + 1])

            # ------- level 8 (scheme A, 4-way split drains -> A9)
            if stop_after < 11:
                return
            A9 = states.tile([128, 512, BC], BF16, tag='state')
            with nc.named_scope('level8'):
                for mg in range(16):  # 16 parents per group
                    if mg % 2 == 0:
                        wtile = wpool.tile([128, 32, 128], BF16, tag='w')
                        nc.sync.dma_start(
                            wtile[:], dr['wa8'].ap()[:, mg * 16:mg * 16 + 32, :])
                    pt = psB.tile([128, 2048], F32, tag='psb')
                    for p in range(16):
                        m = mg * 16 + p
                        tt = (mg % 2) * 16 + p
                        nc.tensor.matmul(
                            pt[:, p * 128:(p + 1) * 128], wtile[:, tt, :],
                            A8[:, m, :, :].rearrange('p b q -> p (b q)'),
                            start=True, stop=True, skip_group_check=True)
                    pv = pt[:].rearrange('p (pr b i) -> p pr b i', pr=16, b=BC)
                    if zero_bias_late:
                        for j in range(2):
                            for i in range(2):
                                dst = A9[i * 64:(i + 1) * 64,
                                         2 * mg * 16 + j: 2 * (mg + 1) * 16: 2, :]
                                drain(dst, pv[j * 64:(j + 1) * 64, :, :, i])
                    else:
                        for p in range(16):
                            m = mg * 16 + p
                            for j in range(2):
                                for i in range(2):
                                    drain(A9[i * 64:(i + 1) * 64, 2 * m + j, :],
                                          pv[j * 64:(j + 1) * 64, p, :, i],
                                          bias=bt[8][j * 64:(j + 1) * 64, m:m + 1])

            # ------- level 9 (scheme A, identity drains -> AF)
            if stop_after < 12:
                return
            AF_t = states.tile([128, 512, BC], BF16, tag='state')
            with nc.named_scope('level9'):
                for mg in range(16):  # 32 parents per group
                    wtile = wpool.tile([128, 32, 128], BF16, tag='w')
                    nc.sync.dma_start(
                        wtile[:], dr['wa9'].ap()[:, mg * 32:mg * 32 + 32, :])
                    pt = psB.tile([128, 2048], F32, tag='psb')
                    for p in range(32):
                        m = mg * 32 + p
                        tt = p
                        nc.tensor.matmul(pt[:, p * 64:(p + 1) * 64],
                                         wtile[:, tt, :], A9[:, m, :],
                                         start=True, stop=True,
                                         skip_group_check=True)
                    if zero_bias_late:
                        drain(AF_t[:, mg * 32:(mg + 1) * 32, :],
                              pt[:].rearrange('p (pr b) -> p pr b', pr=32))
                    else:
                        pv = pt[:].rearrange('p (pr b) -> p pr b', pr=32)
                        for p in range(32):
                            m = mg * 32 + p
                            drain(AF_t[:, m, :], pv[:, p, :],
                                  bias=bt[9][:, m:m + 1])

        # ------- output conv: activations stationary, batch on PSUM partitions
        # psum[b, (pair_local, j', o)] = AF[:, m2, :].T @ wout[:, m2, :]
        if stop_after < 13:
            return
        with tc.tile_pool(name='psO', bufs=4, space='PSUM') as psO:
            with nc.named_scope('out_conv'):
                for mg in range(16):  # 32 pairs per group
                    if mg % 4 == 0:
                        wtile = wpool.tile([128, 128, 16], BF16, tag='w')
                        nc.sync.dma_start(
                            wtile[:], dr['wout'].ap()[:, mg * 32:mg * 32 + 128, :])
                    pt = psO.tile([64, 512], F32, tag='pso')
                    for p in range(32):
                        m2 = mg * 32 + p
                        tt = (mg % 4) * 32 + p
                        nc.tensor.matmul(pt[:, p * 16:(p + 1) * 16],
                                         AF_t[:, m2, :], wtile[:, tt, :],
                                         start=True, stop=True,
                                         skip_group_check=True)
                    ost = wpool.tile([64, 512], F32, tag='ost', bufs=2)
                    drain(ost[:], pt[:], relu=False)
                    nc.sync.dma_start(
                        dr['y'].ap()[:, mg * 512:(mg + 1) * 512], ost[:])


def _scheme_b_level(nc, tc, psA, states, wpool, dr, bt, lvl, Vcur, drain):
    """One scheme-B level: Vcur [128, K/2, BC, L] -> returns [128, K, BC, L/2]."""
    K = 1 << lvl
    L = 1 << (NLVL - lvl)
    Lh = L // 2
    nb = max(1, 512 // Lh)          # batch rows per chunk
    nch = max(1, BC // nb)          # chunks per parent
    cols = nb * Lh
    Vn = states.tile([128, K, BC, Lh], BF16, tag='state')
    wgrp = max(1, min(16, K // 2))  # weight pairs per DMA group
    with nc.named_scope(f'level{lvl}'):
        for tg in range(0, K // 2, wgrp):
            wtile = wpool.tile([128, wgrp, 2, 128], BF16, tag='w')
            nc.sync.dma_start(wtile[:],
                              dr[f'wb{lvl}'].ap()[:, tg:tg + wgrp, :, :])
            for tl in range(wgrp):
                t = tg + tl
                for ch in range(nch):
                    b0 = ch * nb
                    pts = []
                    for s in range(2):  # parent m = 2t+s, rows s*64
                        m = 2 * t + s
                        sp = slice(s * 64, (s + 1) * 64)
                        rhs = Vcur[sp, t, b0:b0 + nb, :] \
                            .rearrange('c b (q i) -> c (b q) i', i=2)
                        pt = psA.tile([128, 512], F32, tag='ps')
                        pts.append(pt)
                        for i in range(2):
                            nc.tensor.matmul(pt[:, 0:cols], wtile[sp, tl, i, :],
                                             rhs[:, :, i],
                                             start=(i == 0), stop=(i == 1),
                                             tile_position=(s * 64, 0))
                    for s in range(2):
                        m = 2 * t + s
                        drain(Vn[:, m, b0:b0 + nb, :],
                              pts[s][:, 0:cols].rearrange(
                                  'p (b q) -> p b q', b=nb),
                              bias=bt[lvl][:, m:m + 1])
    return Vn


# ---------------------------------------------------------------------------
# Public entry point
# ---------------------------------------------------------------------------

_PROGRAM_CACHE = {}


def kernel(in_data, w_in, b_in, filters, biases, mid_w, mid_b, w_out):
    in_data = np.asarray(in_data)
    packed = _pack_weights(w_in, b_in, filters, biases, mid_w, mid_b, w_out)

    zero_bias_late = (
        not np.any(np.asarray(biases, np.float32)[_off(7):])
        and not np.any(np.asarray(mid_b, np.float32)))

    key = ('v1', zero_bias_late)
    if key not in _PROGRAM_CACHE:
        _PROGRAM_CACHE[key] = _build_program(zero_bias_late)
    nc = _PROGRAM_CACHE[key]

    shared = {
        'ident': packed['ident'],
        'win': packed['win'],
        'wa0': packed['wa0'],
        'wmid': packed['wmid'],
        'wa8': packed['wa8'],
        'wa9': packed['wa9'],
        'wout': packed['wout'],
        'b_in': packed['b_in'],
        'bmid': packed['bmid'],
    }
    for lvl in range(1, 8):
        shared[f'wb{lvl}'] = packed[f'wb{lvl}']
    for lvl in range(NLVL):
        shared[f'bl{lvl}'] = packed['bias_levels'][lvl]

    x = np.asarray(in_data, np.float32).reshape(B, 8192)
    in_maps = []
    for c in range(NCORES):
        m = dict(shared)
        m['x'] = x[c * BC:(c + 1) * BC].astype(npbf)
        in_maps.append(m)

    res = bass_utils.run_bass_kernel_spmd(nc, in_maps, core_ids=list(range(NCORES)))
    y = np.concatenate([r['y'] for r in res.results], axis=0)
    return y.reshape(B, 8192, 1).astype(np.float32)


if __name__ == '__main__':
    rng = np.random.default_rng(0)
    pass


# revision 19
# speedup vs baseline: 101.1179x; 101.1179x over previous
"""Butterfly-layer Trainium2 kernel.

Computation (per reference): input conv (8->64ch matmul + relu), 10 butterfly
levels (each branch spawns 2 children via kernel-2 stride-2 conv + relu), a
per-(branch,position) dense CxC layer in the middle, and a per-branch 64->8
output conv.

Sharding: data-parallel over batch across 8 cores (64 rows each), params
replicated.

Layout strategy per core (all activations bf16, fp32 PSUM accumulation):
- State tensors are always [128, 32768] bf16: partition = (half, channel),
  free = (pair, batch, position).
- Levels 1-7 ("scheme B"): contraction over channel (64) with two accumulating
  matmuls (taps i=0,1) reading stride-2 position slices; children pairs land
  stacked on partition halves -> full-width identity PSUM drains.
- Levels 0, 8, 9 ("scheme A"): contraction over (tap, channel) = 128 with both
  children packed in the 128 output columns; needs pair-split input layout
  (position parity on partition halves), produced by 4-way split drains.
- Mid dense: per (branch, position) CxC; branch pairs run as two concurrent
  32x32-granular diagonal tiles (0,0)/(64,64); outputs free-stacked in PSUM.
- Output conv: per branch-pair lhsT [128,16]; results DMA'd from PSUM to DRAM.
"""

import sys
import os

sys.path.insert(0, '/opt/trn_rl_repo')

import numpy as np
import ml_dtypes

import concourse.bass as bass
import concourse.bacc as bacc
import concourse.mybir as mybir
import concourse.tile as tile
from concourse import bass_utils

npbf = ml_dtypes.bfloat16
BF16 = mybir.dt.bfloat16
F32 = mybir.dt.float32
AF = mybir.ActivationFunctionType

NCORES = 8
B = 512
BC = B // NCORES          # 64 batch rows per core
C = 64
IN_F = 8
OUT_F = 8
NLVL = 10
L0 = 1 << NLVL            # 1024
KH = 1 << (NLVL // 2)     # 32
TOTAL_BR = (1 << (NLVL + 1)) - 2  # 2046


def _off(lvl):
    return (1 << (lvl + 1)) - 2


# ---------------------------------------------------------------------------
# Host-side weight packing
# ---------------------------------------------------------------------------

def _pack_weights(w_in, b_in, filters, biases, mid_w, mid_b, w_out):
    f32 = np.float32
    filters = np.asarray(filters, f32)
    w_in = np.asarray(w_in, f32)
    mid_w = np.asarray(mid_w, f32)
    w_out = np.asarray(w_out, f32)
    out = {}

    # identity for PE transpose
    out['ident'] = np.eye(64, dtype=npbf)

    # input conv: 8 block matmuls; lhsT [p=(l16,f)=128, (j,c)=128]
    win = np.zeros((16, 8, 8, 2, 64), f32)  # [l16, f, m, j, c]
    w = w_in[:, 0, :]  # [8, 64]
    for m in range(8):
        for j in range(2):
            win[2 * m + j, :, m, j, :] = w
    out['win'] = win.reshape(128, 8, 128).astype(npbf)

    # level 0 (scheme A, 1 parent): lhsT [(i,c), (j,o)]
    wa0 = filters[0:2].transpose(1, 2, 0, 3).reshape(128, 128)  # [i,c,j,o]
    out['wa0'] = np.ascontiguousarray(wa0).astype(npbf)

    # levels 1..7 (scheme B): [128=(s,c), K/2 pairs, 2 taps, 128=(j,o)]
    for lvl in range(1, 8):
        K = 1 << lvl
        off = _off(lvl)
        f = filters[off:off + 2 * K]          # [2K, 2, C, C] children of level
        # child of parent m, tap i: f[2m+j, i, c, o]
        f = f.reshape(K // 2, 2, 2, 2, C, C)  # [t, s, j, i, c, o]
        wb = f.transpose(1, 4, 0, 3, 2, 5)    # [s, c, t, i, j, o]
        wb = wb.reshape(128, K // 2, 2, 128)
        out[f'wb{lvl}'] = np.ascontiguousarray(wb).astype(npbf)

    # mid: [128=(s,c), 16 t, 32 x, 64 o]
    wm = mid_w.reshape(16, 2, KH, C, C).transpose(1, 3, 0, 2, 4)  # [s,c,t,x,o]
    out['wmid'] = np.ascontiguousarray(wm.reshape(128, 16, KH * C)).astype(npbf)

    # levels 8, 9 (scheme A): [128=(i,c), K parents, 128=(j,o)]
    for lvl in (8, 9):
        K = 1 << lvl
        off = _off(lvl)
        f = filters[off:off + 2 * K].reshape(K, 2, 2, C, C)  # [m, j, i, c, o]
        wa = f.transpose(2, 3, 0, 1, 4).reshape(128, K, 128)  # [(i,c), m, (j,o)]
        out[f'wa{lvl}'] = np.ascontiguousarray(wa).astype(npbf)

    # out conv: [128=(j,c), 512 pairs, 16=(j',o)]
    wo = np.zeros((2, C, 512, 2, OUT_F), f32)  # [j, c, m2, j', o]
    wr = w_out.reshape(512, 2, C, OUT_F)       # [m2, j, c, o]
    for j in range(2):
        wo[j, :, :, j, :] = wr[:, j, :, :].transpose(1, 0, 2)
    out['wout'] = np.ascontiguousarray(wo.reshape(128, 512, 16)).astype(npbf)

    # biases, packed per level as [128, K]: col m = [bias(2m) | bias(2m+1)]
    biases = np.asarray(biases, np.float32)
    bl = {}
    for lvl in range(NLVL):
        K = 1 << lvl
        off = _off(lvl)
        bb = biases[off:off + 2 * K].reshape(K, 2, C).transpose(1, 2, 0)
        bl[lvl] = np.ascontiguousarray(bb.reshape(128, K))
    out['bias_levels'] = bl
    out['b_in'] = np.broadcast_to(np.asarray(b_in, np.float32), (2, 64)).reshape(128, 1).copy()
    # mid bias [128=(s,o), 16 t, 32 x]
    bm = np.asarray(mid_b, np.float32).reshape(16, 2, KH, C).transpose(1, 3, 0, 2)
    out['bmid'] = np.ascontiguousarray(bm.reshape(128, 16, KH))
    return out


# ---------------------------------------------------------------------------
# Bass program
# ---------------------------------------------------------------------------

def _build_program(zero_bias_late, stop_after=99):
    """Build the per-core bass program. zero_bias_late: biases of levels 7-9
    and mid are all-zero -> use pooled (fast) drains there."""
    nc = bacc.Bacc('TRN2', target_bir_lowering=False, debug=False)

    dr = {}
    dr['x'] = nc.dram_tensor('x', [BC, 8192], BF16, kind='ExternalInput')
    dr['ident'] = nc.dram_tensor('ident', [64, 64], BF16, kind='ExternalInput')
    dr['win'] = nc.dram_tensor('win', [128, 8, 128], BF16, kind='ExternalInput')
    dr['wa0'] = nc.dram_tensor('wa0', [128, 128], BF16, kind='ExternalInput')
    for lvl in range(1, 8):
        K = 1 << lvl
        dr[f'wb{lvl}'] = nc.dram_tensor(f'wb{lvl}', [128, K // 2, 2, 128], BF16,
                                        kind='ExternalInput')
    dr['wmid'] = nc.dram_tensor('wmid', [128, 16, KH * C], BF16, kind='ExternalInput')
    dr['wa8'] = nc.dram_tensor('wa8', [128, 256, 128], BF16, kind='ExternalInput')
    dr['wa9'] = nc.dram_tensor('wa9', [128, 512, 128], BF16, kind='ExternalInput')
    dr['wout'] = nc.dram_tensor('wout', [128, 512, 16], BF16, kind='ExternalInput')
    for lvl in range(NLVL):
        dr[f'bl{lvl}'] = nc.dram_tensor(f'bl{lvl}', [128, 1 << lvl], F32,
                                        kind='ExternalInput')
    dr['b_in'] = nc.dram_tensor('b_in', [128, 1], F32, kind='ExternalInput')
    dr['bmid'] = nc.dram_tensor('bmid', [128, 16, KH], F32, kind='ExternalInput')
    dr['y'] = nc.dram_tensor('y', [BC, 8192], F32, kind='ExternalOutput')

    with tile.TileContext(nc) as tc:
        _emit(nc, tc, dr, zero_bias_late, stop_after)
    nc.compile()
    return nc


def _emit(nc, tc, dr, zero_bias_late, stop_after=99):
    from contextlib import ExitStack
    ctx = ExitStack()
    with ctx:
        states = ctx.enter_context(tc.tile_pool(name='state', bufs=2))
        misc = ctx.enter_context(tc.tile_pool(name='misc', bufs=1))
        wpool = ctx.enter_context(tc.tile_pool(name='wts', bufs=3))
        drains = 0  # round-robin ACT/DVE

        def drain(dst, src, bias=None, relu=True):
            nonlocal drains
            drains += 1
            if bias is None and not relu:
                if drains % 2 == 0:
                    nc.scalar.copy(dst, src)
                else:
                    nc.vector.tensor_copy(dst, src)
                return
            if bias is None and relu:
                if drains % 2 == 0:
                    nc.scalar.activation(dst, src, AF.Relu)
                else:
                    nc.vector.tensor_scalar_max(dst, src, 0.0)
                return
            nc.scalar.activation(dst, src, AF.Relu if relu else AF.Copy, bias=bias)

        # constants / biases resident in SBUF
        ident = misc.tile([64, 64], BF16)
        nc.sync.dma_start(ident[:], dr['ident'].ap())
        bt = {}
        for lvl in range(NLVL):
            bt[lvl] = misc.tile([128, 1 << lvl], F32, tag=f'bl{lvl}',
                                name=f'bl{lvl}_t')
            nc.sync.dma_start(bt[lvl][:], dr[f'bl{lvl}'].ap())
        b_in = misc.tile([128, 1], F32, tag='b_in')
        nc.sync.dma_start(b_in[:], dr['b_in'].ap())
        bmid = misc.tile([128, 16, KH], F32, tag='bmid')
        nc.sync.dma_start(bmid[:], dr['bmid'].ap())

        x_in = misc.tile([BC, 8192], BF16, tag='x_in')
        nc.sync.dma_start(x_in[:], dr['x'].ap())

        win = misc.tile([128, 8, 128], BF16, tag='win')
        nc.sync.dma_start(win[:], dr['win'].ap())
        wa0 = misc.tile([128, 128], BF16, tag='wa0')
        nc.sync.dma_start(wa0[:], dr['wa0'].ap())

        xt = misc.tile([128, 64, BC], BF16, tag='xt')  # [(l16,f), lhi, b]

        with tc.tile_pool(name='psA', bufs=6, space='PSUM') as psA:
            # ------- transpose input: 64 chunks of [64,128] -> [128,64]
            if stop_after < 0:
                return
            with nc.named_scope('transpose_in'):
                for g in range(8):
                    pt = psA.tile([128, 512], BF16, tag='ps')
                    for a in range(8):
                        j = g * 8 + a
                        nc.tensor.transpose(pt[:, a * 64:(a + 1) * 64],
                                            x_in[:, j * 128:(j + 1) * 128],
                                            ident[:])
                    drain(xt[:, g * 8:(g + 1) * 8, :], pt[:], relu=False)

            # ------- input conv -> V0 [(i,c), (b, q)]  q = 8*lhi + m
            if stop_after < 1:
                return
            V0 = states.tile([128, BC, 512], BF16, tag='state')
            v0q = V0[:].rearrange('p b (lhi m) -> p lhi b m', m=8)
            with nc.named_scope('in_conv'):
                for m in range(8):
                    for ch in range(8):
                        pt = psA.tile([128, 512], F32, tag='ps')
                        nc.tensor.matmul(pt[:], win[:, m, :],
                                         xt[:, ch * 8:(ch + 1) * 8, :]
                                         .rearrange('p l b -> p (l b)'),
                                         start=True, stop=True)
                        dst = v0q[:, ch * 8:(ch + 1) * 8, :, m]
                        drain(dst, pt[:], bias=b_in[:])

            # ------- level 0 (scheme A, one parent)
            if stop_after < 2:
                return
            V1 = states.tile([128, 1, BC, 512], BF16, tag='state')
            with nc.named_scope('level0'):
                for b0 in range(BC):
                    pt = psA.tile([128, 512], F32, tag='ps')
                    nc.tensor.matmul(pt[:], wa0[:], V0[:, b0, :],
                                     start=True, stop=True)
                    drain(V1[:, 0, b0, :], pt[:], bias=bt[0][:, 0:1])

            # ------- levels 1..4 (scheme B)
            Vcur = V1
            for lvl in range(1, 5):
                if stop_after < 2 + lvl:
                    return
                Vcur = _scheme_b_level(nc, tc, psA, states, wpool, dr, bt, lvl,
                                       Vcur, drain)

            # ------- mid dense (diag tiles)
            if stop_after < 7:
                return
            VM = Vcur  # [128, 16, BC, 32]
            MO = states.tile([128, 16, BC, KH], BF16, tag='state')
            wmt_shape = [128, KH * C]
            with nc.named_scope('mid'):
                for t in range(16):
                    wmt = wpool.tile(wmt_shape, BF16, tag='w')
                    nc.sync.dma_start(wmt[:], dr['wmid'].ap()[:, t, :])
                    wmv = wmt[:].rearrange('p (x o) -> p x o', o=C)
                    for xg in range(4):  # groups of 8 x positions
                        pt = psA.tile([128, 512], F32, tag='ps')
                        for a in range(8):
                            x = xg * 8 + a
                            cs = slice(a * 64, (a + 1) * 64)
                            nc.tensor.matmul(pt[0:64, cs], wmv[0:64, x, :],
                                             VM[0:64, t, :, x],
                                             start=True, stop=True,
                                             skip_group_check=True,
                                             tile_position=(0, 0))
                            nc.tensor.matmul(pt[64:128, cs], wmv[64:128, x, :],
                                             VM[64:128, t, :, x],
                                             start=True, stop=True,
                                             skip_group_check=True,
                                             tile_position=(64, 64))
                        mo_dst = MO[:, t, :, :].rearrange('p b x -> p x b')
                        if zero_bias_late:
                            drain(mo_dst[:, xg * 8:(xg + 1) * 8, :], pt[:])
                        else:
                            for a in range(8):
                                x = xg * 8 + a
                                drain(mo_dst[:, x:x + 1, :],
                                      pt[:, a * 64:(a + 1) * 64]
                                      .rearrange('p c -> p 1 c'),
                                      bias=bmid[:, t, x:x + 1])

            # ------- levels 5, 6 (scheme B)
            Vcur = MO
            for lvl in range(5, 7):
                if stop_after < 3 + lvl:
                    return
                Vcur = _scheme_b_level(nc, tc, psA, states, wpool, dr, bt, lvl,
                                       Vcur, drain)

        # ------- level 7 (scheme B matmuls, 4-way split drains -> A8)
        if stop_after < 10:
            return
        V7 = Vcur  # [128, 64, BC, 8]
        A8 = states.tile([128, 256, BC, 2], BF16, tag='state')
        with tc.tile_pool(name='psB', bufs=2, space='PSUM') as psB:
            with nc.named_scope('level7'):
                # tile_position matmuls require bank-aligned (512-col) psum
                # offsets -> 4 parents per 4-bank tile, 256 cols used each.
                wtile = None
                for mg in range(32):  # 4 parents per group
                    if mg % 4 == 0:
                        wtile = wpool.tile([128, 8, 2, 128], BF16, tag='w')
                        nc.sync.dma_start(
                            wtile[:], dr['wb7'].ap()[:, mg * 2:mg * 2 + 8, :, :])
                    pt = psB.tile([128, 2048], F32, tag='psb')
                    for p in range(4):
                        m = mg * 4 + p
                        t, s = m >> 1, m & 1
                        tt = (mg % 4) * 2 + (p >> 1)
                        sp = slice(s * 64, (s + 1) * 64)
                        rhs = V7[sp, t, :, :].rearrange('c b (q i) -> c b q i', i=2)
                        for i in range(2):
                            nc.tensor.matmul(
                                pt[:, p * 512:p * 512 + 256], wtile[sp, tt, i, :],
                                rhs[:, :, :, i].rearrange('c b q -> c (b q)'),
                                start=(i == 0), stop=(i == 1),
                                skip_group_check=True,
                                tile_position=(s * 64, 0))
                    # drains: psum [(j,o), (4 parents x 512, b, q=2q''+i)] -> A8
                    pv = pt[:].rearrange('p (pr blk) -> p pr blk', pr=4)[:, :, 0:256] \
                        .rearrange('p pr (b q2 i) -> p pr b q2 i', q2=2, i=2)
                    if zero_bias_late:
                        for j in range(2):
                            for i in range(2):
                                dst = A8[i * 64:(i + 1) * 64,
                                         2 * mg * 4 + j: 2 * (mg + 1) * 4: 2, :, :]
                                drain(dst, pv[j * 64:(j + 1) * 64, :, :, :, i])
                    else:
                        for p in range(4):
                            m = mg * 4 + p
                            for j in range(2):
                                for i in range(2):
                                    dst = A8[i * 64:(i + 1) * 64, 2 * m + j, :, :]
                                    drain(dst, pv[j * 64:(j + 1) * 64, p, :, :, i],
                                          bias=bt[7][j * 64:(j + 1) * 64, m:m + 1])

            # ------- level 8 (scheme A, 4-way split drains -> A9)
            if stop_after < 11:
                return
            A9 = states.tile([128, 512, BC], BF16, tag='state')
            with nc.named_scope('level8'):
                for mg in range(16):  # 16 parents per group
                    if mg % 2 == 0:
                        wtile = wpool.tile([128, 32, 128], BF16, tag='w')
                        nc.sync.dma_start(
                            wtile[:], dr['wa8'].ap()[:, mg * 16:mg * 16 + 32, :])
                    pt = psB.tile([128, 2048], F32, tag='psb')
                    for p in range(16):
                        m = mg * 16 + p
                        tt = (mg % 2) * 16 + p
                        nc.tensor.matmul(
                            pt[:, p * 128:(p + 1) * 128], wtile[:, tt, :],
                            A8[:, m, :, :].rearrange('p b q -> p (b q)'),
                            start=True, stop=True, skip_group_check=True)
                    pv = pt[:].rearrange('p (pr b i) -> p pr b i', pr=16, b=BC)
                    if zero_bias_late:
                        for j in range(2):
                            for i in range(2):
                                dst = A9[i * 64:(i + 1) * 64,
                                         2 * mg * 16 + j: 2 * (mg + 1) * 16: 2, :]
                                drain(dst, pv[j * 64:(j + 1) * 64, :, :, i])
                    else:
                        for p in range(16):
                            m = mg * 16 + p
                            for j in range(2):
                                for i in range(2):
                                    drain(A9[i * 64:(i + 1) * 64, 2 * m + j, :],
                                          pv[j * 64:(j + 1) * 64, p, :, i],
                                          bias=bt[8][j * 64:(j + 1) * 64, m:m + 1])

            # ------- level 9 (scheme A, identity drains -> AF)
            if stop_after < 12:
                return
            AF_t = states.tile([128, 512, BC], BF16, tag='state')
            with nc.named_scope('level9'):
                for mg in range(16):  # 32 parents per group
                    wtile = wpool.tile([128, 32, 128], BF16, tag='w')
                    nc.sync.dma_start(
                        wtile[:], dr['wa9'].ap()[:, mg * 32:mg * 32 + 32, :])
                    pt = psB.tile([128, 2048], F32, tag='psb')
                    for p in range(32):
                        m = mg * 32 + p
                        tt = p
                        nc.tensor.matmul(pt[:, p * 64:(p + 1) * 64],
                                         wtile[:, tt, :], A9[:, m, :],
                                         start=True, stop=True,
                                         skip_group_check=True)
                    if zero_bias_late:
                        drain(AF_t[:, mg * 32:(mg + 1) * 32, :],
                              pt[:].rearrange('p (pr b) -> p pr b', pr=32))
                    else:
                        pv = pt[:].rearrange('p (pr b) -> p pr b', pr=32)
                        for p in range(32):
                            m = mg * 32 + p
                            drain(AF_t[:, m, :], pv[:, p, :],
                                  bias=bt[9][:, m:m + 1])

        # ------- output conv: activations stationary, batch on PSUM partitions
        # psum[b, (pair_local, j', o)] = AF[:, m2, :].T @ wout[:, m2, :]
        if stop_after < 13:
            return
        with tc.tile_pool(name='psO', bufs=4, space='PSUM') as psO:
            with nc.named_scope('out_conv'):
                for mg in range(16):  # 32 pairs per group
                    if mg % 4 == 0:
                        wtile = wpool.tile([128, 128, 16], BF16, tag='w')
                        nc.sync.dma_start(
                            wtile[:], dr['wout'].ap()[:, mg * 32:mg * 32 + 128, :])
                    pt = psO.tile([64, 512], F32, tag='pso')
                    for p in range(32):
                        m2 = mg * 32 + p
                        tt = (mg % 4) * 32 + p
                        nc.tensor.matmul(pt[:, p * 16:(p + 1) * 16],
                                         AF_t[:, m2, :], wtile[:, tt, :],
                                         start=True, stop=True,
                                         skip_group_check=True)
                    ost = wpool.tile([64, 512], F32, tag='ost', bufs=2)
                    drain(ost[:], pt[:], relu=False)
                    nc.sync.dma_start(
                        dr['y'].ap()[:, mg * 512:(mg + 1) * 512], ost[:])


def _scheme_b_level(nc, tc, psA, states, wpool, dr, bt, lvl, Vcur, drain):
    """One scheme-B level: Vcur [128, K/2, BC, L] -> returns [128, K, BC, L/2]."""
    K = 1 << lvl
    L = 1 << (NLVL - lvl)
    Lh = L // 2
    nb = max(1, 512 // Lh)          # batch rows per chunk
    nch = max(1, BC // nb)          # chunks per parent
    cols = nb * Lh
    Vn = states.tile([128, K, BC, Lh], BF16, tag='state')
    wgrp = max(1, min(16, K // 2))  # weight pairs per DMA group
    with nc.named_scope(f'level{lvl}'):
        for tg in range(0, K // 2, wgrp):
            wtile = wpool.tile([128, wgrp, 2, 128], BF16, tag='w')
            nc.sync.dma_start(wtile[:],
                              dr[f'wb{lvl}'].ap()[:, tg:tg + wgrp, :, :])
            for tl in range(wgrp):
                t = tg + tl
                for ch in range(nch):
                    b0 = ch * nb
                    pts = []
                    for s in range(2):  # parent m = 2t+s, rows s*64
                        m = 2 * t + s
                        sp = slice(s * 64, (s + 1) * 64)
                        rhs = Vcur[sp, t, b0:b0 + nb, :] \
                            .rearrange('c b (q i) -> c (b q) i', i=2)
                        pt = psA.tile([128, 512], F32, tag='ps')
                        pts.append(pt)
                        for i in range(2):
                            nc.tensor.matmul(pt[:, 0:cols], wtile[sp, tl, i, :],
                                             rhs[:, :, i],
                                             start=(i == 0), stop=(i == 1),
                                             tile_position=(s * 64, 0))
                    for s in range(2):
                        m = 2 * t + s
                        drain(Vn[:, m, b0:b0 + nb, :],
                              pts[s][:, 0:cols].rearrange(
                                  'p (b q) -> p b q', b=nb),
                              bias=bt[lvl][:, m:m + 1])
    return Vn


# ---------------------------------------------------------------------------
# Persistent PJRT runner: caches the jitted executable and device-resident
# replicated inputs so repeat calls only move in_data / outputs.
# ---------------------------------------------------------------------------

class _Runner:
    def __init__(self, nc, n_cores):
        import jax
        from jax.sharding import Mesh, PartitionSpec, NamedSharding
        from jax.experimental.shard_map import shard_map
        from concourse import bass2jax
        import concourse.mybir as _mybir

        bass2jax.install_neuronx_cc_hook()
        self.jax = jax
        self.nc = nc
        self.n_cores = n_cores
        part_name = (nc.partition_id_tensor.name
                     if nc.partition_id_tensor else None)
        in_names, out_names, out_avals, zero_shapes = [], [], [], []
        for alloc in nc.m.functions[0].allocations:
            if not isinstance(alloc, _mybir.MemoryLocationSet):
                continue
            name = alloc.memorylocations[0].name
            if alloc.kind == 'ExternalInput':
                if name != part_name:
                    in_names.append(name)
            elif alloc.kind == 'ExternalOutput':
                shape = tuple(alloc.tensor_shape)
                dtype = _mybir.dt.np(alloc.dtype)
                out_names.append(name)
                out_avals.append(jax.core.ShapedArray(shape, dtype))
                zero_shapes.append((shape, dtype))
        self.param_names = list(in_names)
        self.out_names = out_names
        self.zero_shapes = zero_shapes
        n_params, n_outs = len(in_names), len(out_names)
        all_in = in_names + out_names + ([part_name] if part_name else [])

        devices = jax.devices()[:n_cores]
        self.mesh = Mesh(np.array(devices), ('core',))
        self.sharding = NamedSharding(self.mesh, PartitionSpec('core'))
        in_specs = (PartitionSpec('core'),) * (n_params + n_outs)
        out_specs = (PartitionSpec('core'),) * n_outs

        def _body(*args):
            operands = list(args)
            if part_name is not None:
                operands.append(bass2jax.partition_id_tensor())
            outs = bass2jax._bass_exec_p.bind(
                *operands, out_avals=tuple(out_avals),
                in_names=tuple(all_in), out_names=tuple(out_names),
                lowering_input_output_aliases=(),
                sim_require_finite=True, sim_require_nnan=True, nc=nc)
            return tuple(outs)

        donate = tuple(range(n_params, n_params + n_outs))
        self.fn = jax.jit(
            shard_map(_body, mesh=self.mesh, in_specs=in_specs,
                      out_specs=out_specs, check_rep=False),
            donate_argnums=donate, keep_unused=True)
        self._dev_cache = {}

    def _dev(self, name, arrays):
        """Concat per-core arrays and cache on device, keyed by array ids."""
        key = (name, tuple(id(a) for a in arrays))
        hit = self._dev_cache.get(name)
        if hit is not None and hit[0] == key:
            return hit[1]
        cat = np.concatenate([np.asarray(a) for a in arrays], axis=0)
        darr = self.jax.device_put(cat, self.sharding)
        darr.block_until_ready()
        self._dev_cache[name] = (key, darr)
        return darr

    def _zeros(self):
        import jax.numpy as jnp
        outs = []
        for shape, dtype in self.zero_shapes:
            gshape = (self.n_cores * shape[0],) + tuple(shape[1:])
            outs.append(self.jax.device_put(
                jnp.zeros(gshape, dtype), self.sharding))
        return outs

    def run_device(self, in_maps):
        """Returns list of per-output global device arrays."""
        args = [self._dev(n, [m[n] for m in in_maps]) for n in self.param_names]
        return self.fn(*args, *self._zeros())

    def __call__(self, in_maps):
        out_arrs = self.run_device(in_maps)
        res = []
        for c in range(self.n_cores):
            res.append({
                name: np.asarray(out_arrs[i])[c * self.zero_shapes[i][0][0]:
                                              (c + 1) * self.zero_shapes[i][0][0]]
                for i, name in enumerate(self.out_names)})
        return res


# ---------------------------------------------------------------------------
# Public entry point
# ---------------------------------------------------------------------------

_PROGRAM_CACHE = {}


def kernel(in_data, w_in, b_in, filters, biases, mid_w, mid_b, w_out):
    in_data = np.asarray(in_data)
    packed = _pack_weights(w_in, b_in, filters, biases, mid_w, mid_b, w_out)

    zero_bias_late = (
        not np.any(np.asarray(biases, np.float32)[_off(7):])
        and not np.any(np.asarray(mid_b, np.float32)))

    key = ('v1', zero_bias_late)
    if key not in _PROGRAM_CACHE:
        nc = _build_program(zero_bias_late)
        _PROGRAM_CACHE[key] = _Runner(nc, NCORES)
    runner = _PROGRAM_CACHE[key]

    shared = {
        'ident': packed['ident'],
        'win': packed['win'],
        'wa0': packed['wa0'],
        'wmid': packed['wmid'],
        'wa8': packed['wa8'],
        'wa9': packed['wa9'],
        'wout': packed['wout'],
        'b_in': packed['b_in'],
        'bmid': packed['bmid'],
    }
    for lvl in range(1, 8):
        shared[f'wb{lvl}'] = packed[f'wb{lvl}']
    for lvl in range(NLVL):
        shared[f'bl{lvl}'] = packed['bias_levels'][lvl]

    x = np.asarray(in_data, np.float32).reshape(B, 8192)
    in_maps = []
    for c in range(NCORES):
        m = dict(shared)
        m['x'] = x[c * BC:(c + 1) * BC].astype(npbf)
        in_maps.append(m)

    res = runner(in_maps)
    y = np.concatenate([r['y'] for r in res], axis=0)
    return y.reshape(B, 8192, 1).astype(np.float32)


if __name__ == '__main__':
    rng = np.random.default_rng(0)
    pass
